# revision 49
# baseline (speedup 1.0000x reference)
"""Trainium2 Bass kernel for nn_CLIP_GCN_Model (2-layer GCN + MLP + contrastive loss).

Reformulation (validated numerically): out = mean_i(label_i * (lse_i - logits_ii)) + 1.0
(the triplet term of the reference is identically 1.0).

GCN layer: out = S @ (x @ W) + b where S = D^-1/2 (A+I) D^-1/2.
  Layer 1: gather raw x rows (512-wide) per dst-chunk, aggregate via one-hot
    coefficient matmuls (C_j.T @ rows_j accumulated in PSUM), then apply W_g1.
  Layer 2: only the ~3.3k unique labeled nodes need gcn_out. Each core owns a
    balanced subset (LCH chunks of 128); gathers h rows (256-wide), aggregates,
    applies W_g2. Edges are split by AllGather-half of their src so the first
    gather of each chunk can start after AG half 1.

Sharding: 80 L1 dst-chunks assigned 10/core by LPT on edge counts (balances the
work in front of each AllGather). h AllGathered in 2 halves (bubble filled by
the image MLP). Labeled nodes assigned to cores by LPT on batch-row counts.
Contrastive row r lives on the core owning label_r; image columns permuted per
core so owned rows' diagonal lands at local col == local row. Gathers spread
round-robin over the 4 SWDGE queues (one Q7 core pair each) for 4x descriptor
emission concurrency.
"""

import os
import numpy as np
import ml_dtypes

BF16 = ml_dtypes.bfloat16

N_NODES = 10000
NPAD = 10240
D = 512
Hdim = 256
BATCH = 4096
NCORES = 8
P = 128
NCHUNK = NPAD // P          # 80
CPC = NCHUNK // NCORES      # 10 chunks per core
NPC = NPAD // NCORES        # 1280 nodes per core (h rows per core in h_t)
NT = BATCH // 512           # 8 column tiles of 512
CH1 = CPC // 2              # chunks before AllGather half 1


def _wrap16(idx, n):
    """Layout indices for dma_gather: element i -> [i%16, i//16], replicated to 128 partitions."""
    assert len(idx) == n and n % 16 == 0
    base = idx.astype(np.int16).reshape(n // 16, 16).T  # [16, n/16]
    return np.ascontiguousarray(np.tile(base, (8, 1)))  # [128, n/16]


def _lpt_assign(weights, n_bins, cap):
    """Greedy LPT: assign items (sorted by weight desc) to least-loaded bin with
    < cap items. Returns list of item-index lists per bin."""
    order = np.argsort(-np.asarray(weights), kind="stable")
    loads = [0.0] * n_bins
    counts = [0] * n_bins
    bins = [[] for _ in range(n_bins)]
    for it in order:
        best = min((b for b in range(n_bins) if counts[b] < cap),
                   key=lambda b: loads[b])
        bins[best].append(int(it))
        loads[best] += weights[it]
        counts[best] += 1
    return bins


def _prep(inputs):
    """Host-side layout/sharding prep. Returns (shared, percore, dims)."""
    x = np.ascontiguousarray(np.asarray(inputs["x_nodes"], dtype=np.float32))
    image = np.ascontiguousarray(np.asarray(inputs["image"], dtype=np.float32))
    ei = np.asarray(inputs["edge_index"]).astype(np.int64)
    label = np.asarray(inputs["label"]).astype(np.int64)
    src, dst = ei[0], ei[1]

    deg = np.ones(N_NODES, np.float32)
    np.add.at(deg, dst, 1.0)
    dinv = (1.0 / np.sqrt(deg)).astype(np.float32)

    # edges + self loops, sorted by dst
    src_all = np.concatenate([src, np.arange(N_NODES)])
    dst_all = np.concatenate([dst, np.arange(N_NODES)])
    coef_all = np.concatenate([dinv[src] * dinv[dst], dinv * dinv]).astype(np.float32)
    order = np.argsort(dst_all, kind="stable")
    src_s, dst_s, coef_s = src_all[order], dst_all[order], coef_all[order]

    counts = np.bincount(dst_s // P, minlength=NCHUNK)
    T_MAX = int(np.ceil(counts.max() / P))
    E_c = T_MAX * P
    starts = np.zeros(NCHUNK + 1, np.int64)
    np.cumsum(counts, out=starts[1:])

    # ---- L1 chunk -> core assignment, balanced by edge count ----
    chunk_bins = _lpt_assign(counts.astype(np.float64), NCORES, CPC)
    # remap: global node -> row in h_t ([half][core][pos%5][128])
    chunk_core = np.zeros(NCHUNK, np.int64)
    chunk_pos = np.zeros(NCHUNK, np.int64)
    for c in range(NCORES):
        for p_i, g in enumerate(chunk_bins[c]):
            chunk_core[g] = c
            chunk_pos[g] = p_i
    # single post-L1 AllGather: h_t rows are [core][pos][128]
    n_all = np.arange(NPAD)
    cg = n_all // P
    remap_row = chunk_core[cg] * NPC + chunk_pos[cg] * P + (n_all % P)

    # ---- per-core L1 gather/coefficient structures ----
    # L1 gathers read the on-device-computed xw1 tables (xwA = nodes < 5120,
    # xwB = rest), stored partition-interleaved: node n -> row (n%128)*40 + n//128
    # within its half. Edges dedup by src within a chunk and split by half so
    # the A gathers only depend on the xwA stores.
    HN = NPAD // 2
    HT = HN // P  # 40 tiles per half

    def _xw_idx(u):
        u = u % HN
        return (u % P) * HT + u // P

    chunk_data = {}
    cntA = np.zeros(NCHUNK, np.int64)
    cntB = np.zeros(NCHUNK, np.int64)
    for g in range(NCHUNK):
        e0, e1 = starts[g], starts[g + 1]
        usrc, inv = np.unique(src_s[e0:e1], return_inverse=True)
        nA = int((usrc < HN).sum())
        chunk_data[g] = (usrc, inv, dst_s[e0:e1] - g * P, coef_s[e0:e1], nA)
        cntA[g], cntB[g] = nA, len(usrc) - nA
    JA1 = int(np.ceil(cntA.max() / P))
    JB1 = int(np.ceil(cntB.max() / P))
    T_MAX = JA1 + JB1
    E_c = T_MAX * P

    l1 = []
    for c in range(NCORES):
        gidx = np.zeros((CPC, E_c), np.int64)
        C = np.zeros((CPC, E_c, P), np.float32)
        for i, g in enumerate(chunk_bins[c]):
            usrc, inv, ld, cf, nA = chunk_data[g]
            # unique srcs are sorted, so [:nA] is half A
            gidx[i, :nA] = _xw_idx(usrc[:nA])
            gidx[i, JA1 * P:JA1 * P + len(usrc) - nA] = _xw_idx(usrc[nA:])
            # row position of each edge's (deduped) src in the tile layout
            rowpos = np.where(inv < nA, inv, JA1 * P + inv - nA)
            np.add.at(C[i], (rowpos, ld), cf)
        C = C.reshape(CPC, T_MAX, P, P).astype(BF16)
        l1.append({"gidx": gidx, "C": C})

    # ---- labeled-node ownership, balanced by batch-row count ----
    uniq, lab_inv, lab_rows = np.unique(label, return_inverse=True,
                                        return_counts=True)
    n_uniq = len(uniq)
    LCH = int(np.ceil(np.ceil(n_uniq / NCORES) / P))
    CAP = LCH * P
    node_bins = _lpt_assign(lab_rows.astype(np.float64), NCORES, CAP)
    LCH = int(np.ceil(max(len(b) for b in node_bins) / P))
    CAP = LCH * P

    # within each core, deal nodes round-robin by degree into LCH chunks so
    # per-chunk edge counts stay even (T2 is a global max)
    own_nodes = np.full((NCORES, CAP), -1, np.int64)   # global node id per slot
    owner_of = {}
    slot_of = {}
    for c in range(NCORES):
        items = node_bins[c]
        # true edge count of node u = edges with dst == u (computed below per
        # node is expensive; chunk-level degree is a fine proxy for dealing)
        degs = np.asarray([deg[uniq[i]] for i in items])
        items = [items[k] for k in np.argsort(-degs, kind="stable")]
        per_chunk = [[] for _ in range(LCH)]
        for k, it in enumerate(items):
            per_chunk[k % LCH].append(it)
        for ch in range(LCH):
            assert len(per_chunk[ch]) <= P
            for d, it in enumerate(per_chunk[ch]):
                u = int(uniq[it])
                own_nodes[c, ch * P + d] = u
                owner_of[u] = c
                slot_of[u] = ch * P + d

    # ---- per-core L2 structures (labeled chunks, src-half-split edges) ----
    # first pass: per (core, chunk) collect edges split by src half
    edgesA = [[[] for _ in range(LCH)] for _ in range(NCORES)]
    edgesB = [[[] for _ in range(LCH)] for _ in range(NCORES)]
    for c in range(NCORES):
        for s in range(CAP):
            u = own_nodes[c, s]
            if u < 0:
                continue
            ch, d = s // P, s % P
            e0, e1 = starts[u // P], starts[u // P + 1]
            sel = dst_s[e0:e1] == u
            esrc = src_s[e0:e1][sel]
            ecoef = coef_s[e0:e1][sel]
            rows = remap_row[esrc]
            for r, cf in zip(rows, ecoef):
                (edgesA if r < NPAD // 2 else edgesB)[c][ch].append((int(r), d, float(cf)))
    JA = max(1, int(np.ceil(max(len(edgesA[c][ch]) for c in range(NCORES)
                                for ch in range(LCH)) / P)))
    JB = max(1, int(np.ceil(max(len(edgesB[c][ch]) for c in range(NCORES)
                                for ch in range(LCH)) / P)))
    T2 = JA + JB
    E2 = T2 * P

    l2 = []
    for c in range(NCORES):
        # half-A rows index h_t_a directly; half-B rows are rebased to h_t_b
        # (row - NPAD//2). Padding rows stay 0 (valid in either half-table).
        gidx2 = np.zeros((LCH, E2), np.int64)
        ldst2 = np.zeros((LCH, E2), np.int64)
        cval2 = np.zeros((LCH, E2), np.float32)
        for ch in range(LCH):
            ea, eb = edgesA[c][ch], edgesB[c][ch]
            for k, (r, d, cf) in enumerate(ea):
                gidx2[ch, k] = r
                ldst2[ch, k] = d
                cval2[ch, k] = cf
            for k, (r, d, cf) in enumerate(eb):
                gidx2[ch, JA * P + k] = r - NPAD // 2
                ldst2[ch, JA * P + k] = d
                cval2[ch, JA * P + k] = cf
        C2 = np.zeros((LCH, T2, P, P), BF16)
        jj = np.arange(E2) // P
        pp = np.arange(E2) % P
        for ch in range(LCH):
            C2[ch, jj, pp, ldst2[ch]] = cval2[ch].astype(BF16)
        l2.append({"gidx2": gidx2, "C2": C2})

    xpad = np.zeros((NPAD, D), np.float32)
    xpad[:N_NODES] = x
    # x.T in matmul lhsT layout: [128 k-part, 4 k-tiles, 10240 nodes]
    xt = np.ascontiguousarray(
        xpad.T.reshape(4, P, NPAD).transpose(1, 0, 2)).astype(BF16)

    def km(w, kt):  # [K, M] -> [128p, kt, M]
        return np.ascontiguousarray(
            w.reshape(kt, P, w.shape[1]).transpose(1, 0, 2)
        ).astype(BF16)

    shared = {
        "xt": xt,
        "wg1": km(np.asarray(inputs["W_g1"], np.float32), 4),   # [128, 4, 256]
        "wg2": km(np.asarray(inputs["W_g2"], np.float32), 2),   # [128, 2, 512]
        "wi1": np.ascontiguousarray(
            np.asarray(inputs["W_img1"], np.float32).reshape(4, P, 2, P).transpose(1, 0, 2, 3)
        ).astype(BF16),                                         # [128, 4k, 2m, 128]
        "wi2": np.ascontiguousarray(
            np.asarray(inputs["W_img2"], np.float32).reshape(2, P, 4, P).transpose(1, 0, 2, 3)
        ).astype(BF16),                                         # [128, 2k, 4m, 128]
        "bg1": np.asarray(inputs["b_g1"], np.float32).astype(BF16).reshape(1, Hdim),
        "bg2": np.asarray(inputs["b_g2"], np.float32).astype(BF16).reshape(1, D),
        "bi1": np.ascontiguousarray(np.asarray(inputs["b_img1"], np.float32).reshape(2, P).T),
        "bi2": np.ascontiguousarray(np.asarray(inputs["b_img2"], np.float32).reshape(4, P).T),
    }

    # ---- contrastive rows: owner core of each batch row's label ----
    # rows ordered by the label's L2 chunk so logits row-tile r only needs
    # g_own chunks 0..KR[r] (enables L2/logits pipelining)
    owner = np.array([owner_of[int(l)] for l in label], np.int64)
    rows_by_core = []
    for c in range(NCORES):
        r = np.where(owner == c)[0]
        ch_of = np.array([slot_of[int(l)] // P for l in label[r]])
        rows_by_core.append(r[np.argsort(ch_of, kind="stable")])
    RT = max(2, int(np.ceil(max(len(r) for r in rows_by_core) / P)))
    ROWS = RT * P
    # KR[r] = max L2 chunk needed by logits row-tile r (global max over cores,
    # nondecreasing; pad rows use slot 0 = chunk 0)
    KR = [0] * RT
    for c in range(NCORES):
        slots = np.zeros(ROWS, np.int64)
        rl = rows_by_core[c]
        slots[:len(rl)] = [slot_of[int(l)] for l in label[rl]]
        for r in range(RT):
            KR[r] = max(KR[r], int(slots[r * P:(r + 1) * P].max()) // P)
    for r in range(1, RT):
        KR[r] = max(KR[r], KR[r - 1])
    KR = tuple(KR)

    percore = []
    imageb = image.astype(BF16)
    for c in range(NCORES):
        Cc = np.ascontiguousarray(
            l1[c]["C"].transpose(2, 0, 1, 3))          # [128p, 10, T_MAX, 128]
        gi = np.zeros((P, CPC, E_c // 16), np.int16)
        for i in range(CPC):
            gi[:, i, :] = _wrap16(l1[c]["gidx"][i], E_c)
        C2c = np.ascontiguousarray(
            l2[c]["C2"].transpose(2, 0, 1, 3))         # [128p, LCH, T2, 128]
        gi2 = np.zeros((P, LCH, E2 // 16), np.int16)
        for ch in range(LCH):
            gi2[:, ch, :] = _wrap16(l2[c]["gidx2"][ch], E2)

        rows = rows_by_core[c]
        n_c = len(rows)
        loc = np.zeros(ROWS, np.int64)
        loc[:n_c] = [slot_of[int(l)] for l in label[rows]]
        li = _wrap16(loc, ROWS)
        lab_f = np.zeros(ROWS, np.float32)
        lab_f[:n_c] = label[rows].astype(np.float32)
        lab_f = np.ascontiguousarray(lab_f.reshape(RT, P).T)   # [128, RT]
        rest = np.setdiff1d(np.arange(BATCH), rows, assume_unique=True)
        perm = np.concatenate([rows, rest])
        imr = imageb[perm]
        imt = np.ascontiguousarray(
            imr.reshape(NT, 512, 4, P).transpose(0, 3, 2, 1)
        )  # [8, 128p, 4k, 512w]
        percore.append({"cmat": Cc, "gidx": gi, "cmat2": C2c, "gidx2": gi2,
                        "lidx": li, "labf": lab_f, "imt": imt,
                        "own_nodes": own_nodes[c], "rows": rows})
    dims = dict(T_MAX=T_MAX, JA1=JA1, T2=T2, JA=JA, LCH=LCH, RT=RT, KR=KR)
    return shared, percore, dims


def _build(T_MAX, JA1, T2, JA, LCH, RT, KR):
    """Build the SPMD Bass program."""
    import concourse.bass as bass  # noqa: F401
    import concourse.tile as tile
    from concourse import bacc, mybir
    from concourse.masks import make_identity

    fp32 = mybir.dt.float32
    bf16 = mybir.dt.bfloat16
    i16 = mybir.dt.int16
    AF = mybir.ActivationFunctionType
    AX = mybir.AxisListType
    E_c = T_MAX * P
    E2 = T2 * P
    JB = T2 - JA
    ROWS = RT * P
    LROWS = LCH * P

    nc = bacc.Bacc("TRN2", target_bir_lowering=False, debug=False,
                   num_devices=NCORES, num_swdge_queues=4)

    t_xt = nc.dram_tensor("xt", [P, 4, NPAD], bf16, kind="ExternalInput").ap()
    t_wg1 = nc.dram_tensor("wg1", [P, 4, Hdim], bf16, kind="ExternalInput").ap()
    t_wg2 = nc.dram_tensor("wg2", [P, 2, D], bf16, kind="ExternalInput").ap()
    t_wi1 = nc.dram_tensor("wi1", [P, 4, 2, P], bf16, kind="ExternalInput").ap()
    t_wi2 = nc.dram_tensor("wi2", [P, 2, 4, P], bf16, kind="ExternalInput").ap()
    t_bg1 = nc.dram_tensor("bg1", [1, Hdim], bf16, kind="ExternalInput").ap()
    t_bg2 = nc.dram_tensor("bg2", [1, D], bf16, kind="ExternalInput").ap()
    t_bi1 = nc.dram_tensor("bi1", [P, 2], fp32, kind="ExternalInput").ap()
    t_bi2 = nc.dram_tensor("bi2", [P, 4], fp32, kind="ExternalInput").ap()
    t_cmat = nc.dram_tensor("cmat", [P, CPC, T_MAX, P], bf16, kind="ExternalInput").ap()
    t_gidx = nc.dram_tensor("gidx", [P, CPC, E_c // 16], i16, kind="ExternalInput").ap()
    t_cmat2 = nc.dram_tensor("cmat2", [P, LCH, T2, P], bf16, kind="ExternalInput").ap()
    t_gidx2 = nc.dram_tensor("gidx2", [P, LCH, E2 // 16], i16, kind="ExternalInput").ap()
    t_lidx = nc.dram_tensor("lidx", [P, ROWS // 16], i16, kind="ExternalInput").ap()
    t_labf = nc.dram_tensor("labf", [P, RT], fp32, kind="ExternalInput").ap()
    t_imt = nc.dram_tensor("imt", [NT, P, 4, 512], bf16, kind="ExternalInput").ap()
    t_out = nc.dram_tensor("partial", [1, 1], fp32, kind="ExternalOutput").ap()

    rg = [list(range(NCORES))]

    with tile.TileContext(nc) as tc:
        from contextlib import ExitStack
        with ExitStack() as ctx:
            dram = ctx.enter_context(tc.tile_pool(name="dram", bufs=1, space="DRAM"))
            const = ctx.enter_context(tc.tile_pool(name="const", bufs=1))
            big = ctx.enter_context(tc.tile_pool(name="big", bufs=1))
            work = ctx.enter_context(tc.tile_pool(name="work", bufs=3))
            gbuf = ctx.enter_context(tc.tile_pool(name="gbuf", bufs=2))
            cbuf = ctx.enter_context(tc.tile_pool(name="cbuf", bufs=3))
            stat = ctx.enter_context(tc.tile_pool(name="stat", bufs=4))

            h_own = dram.tile([NPC, Hdim], bf16)
            h_t = dram.tile([NPAD, Hdim], bf16, addr_space="Shared")
            HT = NPAD // 2 // P  # 40 xw1 tiles per half
            xwA = dram.tile([P, HT, Hdim], bf16)   # xw1 rows, half A interleaved
            xwB = dram.tile([P, HT, Hdim], bf16)

            # ---- constants in SBUF (gather indices first: gathers need them) ----
            gidx_s = const.tile([P, CPC, E_c // 16], i16)
            nc.sync.dma_start(out=gidx_s[:], in_=t_gidx[:])

            imgT_s = big.tile([P, 4, BATCH], bf16)  # transposed img (permuted cols)
            g_sball = big.tile([P, LCH, D], bf16)   # gcn_out: slot s -> [s%128, s//128, :]

            # ---- constants ----
            wg1_s = const.tile([P, 4, Hdim], bf16)
            nc.sync.dma_start(out=wg1_s[:], in_=t_wg1[:])
            bg1_s = const.tile([1, Hdim], bf16)
            nc.sync.dma_start(out=bg1_s[:], in_=t_bg1[:])
            wi1_s = const.tile([P, 4, 2, P], bf16)
            nc.scalar.dma_start(out=wi1_s[:], in_=t_wi1[:])
            wi2_s = const.tile([P, 2, 4, P], bf16)
            nc.scalar.dma_start(out=wi2_s[:], in_=t_wi2[:])
            bi1_s = const.tile([P, 2], fp32)
            nc.scalar.dma_start(out=bi1_s[:], in_=t_bi1[:])
            bi2_s = const.tile([P, 4], fp32)
            nc.scalar.dma_start(out=bi2_s[:], in_=t_bi2[:])
            wg2_s = const.tile([P, 2, D], bf16)
            nc.sync.dma_start(out=wg2_s[:], in_=t_wg2[:])
            bg2_s = const.tile([1, D], bf16)
            nc.sync.dma_start(out=bg2_s[:], in_=t_bg2[:])
            gidx2_s = const.tile([P, LCH, E2 // 16], i16)
            nc.sync.dma_start(out=gidx2_s[:], in_=t_gidx2[:])
            lidx_s = const.tile([P, ROWS // 16], i16)
            nc.sync.dma_start(out=lidx_s[:], in_=t_lidx[:])
            labf_s = const.tile([P, RT], fp32)
            nc.sync.dma_start(out=labf_s[:], in_=t_labf[:])
            ones_row = const.tile([1, P], bf16)
            nc.vector.memset(ones_row[:], 1.0)
            ones_col = const.tile([P, 1], fp32)
            nc.vector.memset(ones_col[:], 1.0)
            ident_b = const.tile([P, P], bf16)
            make_identity(nc, ident_b[:])
            ident_f = const.tile([P, P], fp32)
            make_identity(nc, ident_f[:])

            # ===== replicated xw1 = x @ W_g1, written to xwA/xwB interleaved =
            with tc.tile_pool(name="ps_pre", bufs=2, space="PSUM") as ps_pre:
                for grp in range(8):
                    xt_g = work.tile([P, 4, NPC], bf16, tag="xt_g", name="xt_g",
                                     bufs=2)
                    nc.sync.dma_start(
                        out=xt_g[:], in_=t_xt[:, :, grp * NPC:(grp + 1) * NPC])
                    xw_sb = work.tile([P, CPC, Hdim], bf16, tag="xw_sb",
                                      name="xw_sb", bufs=2)
                    for tl in range(CPC):
                        pw = ps_pre.tile([P, Hdim], fp32, tag="xw", name="pw")
                        for k in range(4):
                            nc.tensor.matmul(
                                out=pw[:], lhsT=xt_g[:, k, tl * P:(tl + 1) * P],
                                rhs=wg1_s[:, k, :], start=(k == 0), stop=(k == 3),
                            )
                        nc.vector.tensor_copy(out=xw_sb[:, tl, :], in_=pw[:])
                    half, gg = (xwA, grp) if grp < 4 else (xwB, grp - 4)
                    nc.sync.dma_start(
                        out=half[:, gg * CPC:(gg + 1) * CPC, :], in_=xw_sb[:])

            # ===== L1 prefetch: cm load (sync) + half-split gathers ==========
            JB1 = T_MAX - JA1
            fetched = {}

            def l1_fetch(i):
                cm = cbuf.tile([P, T_MAX, P], bf16, tag="cm", name="cm")
                nc.sync.dma_start(out=cm[:], in_=t_cmat[:, i, :, :])
                ghs = []
                for hf, (tbl, j0, jn) in enumerate(
                        ((xwA, 0, JA1), (xwB, JA1, JB1))):
                    gh = gbuf.tile([P, jn, Hdim], bf16, tag=f"g1_{hf}", name="gh")
                    nc.gpsimd.dma_gather(
                        out_ap=gh[:], in_ap=tbl[0, :, :],
                        idxs_ap=gidx_s[:, i, j0 * 8:(j0 + jn) * 8],
                        num_idxs=jn * P, num_idxs_reg=jn * P,
                        elem_size=Hdim, single_packet=False,
                        queue_num=(2 * i + hf) % 4,
                    )
                    ghs.append(gh)
                fetched[i] = (cm, ghs)

            l1_fetch(0)
            l1_fetch(1)

            # ---- image MLP tile (used at start for PE warmup + AG2 bubble) ----
            def mlp_tile(n, ps_mlp):
                imt_n = work.tile([P, 4, 512], bf16, tag="imt_n", name="imt_n", bufs=2)
                nc.scalar.dma_start(out=imt_n[:], in_=t_imt[n])
                h1t = work.tile([P, 2, 512], bf16, tag="h1t", name="h1t")
                for m in range(2):
                    pm = ps_mlp.tile([P, 512], fp32, tag="mlp1", name="pm")
                    for k in range(4):
                        nc.tensor.matmul(
                            out=pm[:], lhsT=wi1_s[:, k, m, :], rhs=imt_n[:, k, :],
                            start=(k == 0), stop=(k == 3),
                        )
                    nc.scalar.activation(
                        out=h1t[:, m, :], in_=pm[:], func=AF.Relu,
                        bias=bi1_s[:, m:m + 1], scale=1.0,
                    )
                for m in range(4):
                    pm2 = ps_mlp.tile([P, 512], fp32, tag="mlp2", name="pm2")
                    for k in range(2):
                        nc.tensor.matmul(
                            out=pm2[:], lhsT=wi2_s[:, k, m, :], rhs=h1t[:, k, :],
                            start=(k == 0), stop=(k == 1),
                        )
                    nc.scalar.activation(
                        out=imgT_s[:, m, n * 512:(n + 1) * 512], in_=pm2[:],
                        func=AF.Relu, bias=bi2_s[:, m:m + 1], scale=1.0,
                    )

            # ===== GCN layer 1 compute (consumes prefetched cm + gathers) =====
            def l1_compute(i, ps_ag):
                cm, ghs = fetched.pop(i)
                pa = ps_ag.tile([P, Hdim], fp32, tag="agg1", name="pa")
                jsp = ((0, JA1), (JA1, JB1))
                for hf in range(2):
                    j0, jn = jsp[hf]
                    for j in range(jn):
                        nc.tensor.matmul(
                            out=pa[:], lhsT=cm[:, j0 + j, :], rhs=ghs[hf][:, j, :],
                            start=(hf == 0 and j == 0), stop=False,
                        )
                nc.tensor.matmul(
                    out=pa[:], lhsT=ones_row[:], rhs=bg1_s[:],
                    start=False, stop=True, skip_group_check=True,
                )
                h_sb = work.tile([P, Hdim], bf16, tag="h_sb", name="h_sb")
                nc.vector.tensor_scalar_max(out=h_sb[:], in0=pa[:], scalar1=0.0)
                nc.sync.dma_start(out=h_own[i * P:(i + 1) * P, :], in_=h_sb[:])

            with tc.tile_pool(name="ps_ag", bufs=2, space="PSUM") as ps_ag:
                for i in range(CPC):
                    if i + 2 < CPC:
                        l1_fetch(i + 2)
                    l1_compute(i, ps_ag)
                nc.gpsimd.collective_compute(
                    "AllGather", mybir.AluOpType.bypass, replica_groups=rg,
                    ins=[h_own[:, :]], outs=[h_t[:, :]],
                )
                # the whole image MLP fills the AllGather bubble and keeps the
                # PE warm into the L2/logits tail
                for n in range(NT):
                    mlp_tile(n, ps_ag)

            # ====== L2 gathers: early halves (need AG1) then late (need AG2) ==
            # emitted on the gpsimd queue before the MLP tensor work so the Q7
            # pairs stay busy through the AG2 bubble.
            g2e, g2l, cm2s = [], [], []
            for ch in range(LCH):
                cm2 = cbuf.tile([P, T2, P], bf16, tag="cm2", name="cm2", bufs=LCH)
                nc.sync.dma_start(out=cm2[:], in_=t_cmat2[:, ch, :, :])
                cm2s.append(cm2)
            for ch in range(LCH):
                ge = gbuf.tile([P, JA, Hdim], bf16, tag="g2e", name="ge", bufs=LCH)
                nc.gpsimd.dma_gather(
                    out_ap=ge[:], in_ap=h_t[0:NPAD // 2, :],
                    idxs_ap=gidx2_s[:, ch, 0:JA * 8],
                    num_idxs=JA * P, num_idxs_reg=JA * P,
                    elem_size=Hdim, single_packet=False,
                    queue_num=ch % 4,
                )
                g2e.append(ge)
            for ch in range(LCH):
                gl = gbuf.tile([P, JB, Hdim], bf16, tag="g2l", name="gl", bufs=LCH)
                nc.gpsimd.dma_gather(
                    out_ap=gl[:], in_ap=h_t[NPAD // 2:NPAD, :],
                    idxs_ap=gidx2_s[:, ch, JA * 8:T2 * 8],
                    num_idxs=JB * P, num_idxs_reg=JB * P,
                    elem_size=Hdim, single_packet=False,
                    queue_num=(ch + LCH) % 4,
                )
                g2l.append(gl)

            # ====== GCN layer 2 + txt + logits, pipelined per KR schedule =====
            def l2_chunk(ch, ps_l2):
                pa2 = ps_l2.tile([P, Hdim], fp32, tag="agg2", name="pa2")
                for j in range(JA):
                    nc.tensor.matmul(
                        out=pa2[:], lhsT=cm2s[ch][:, j, :], rhs=g2e[ch][:, j, :],
                        start=(j == 0), stop=False,
                    )
                for j in range(JB):
                    nc.tensor.matmul(
                        out=pa2[:], lhsT=cm2s[ch][:, JA + j, :], rhs=g2l[ch][:, j, :],
                        start=False, stop=(j == JB - 1),
                    )
                a2 = work.tile([P, Hdim], bf16, tag="a2", name="a2")
                nc.vector.tensor_copy(out=a2[:], in_=pa2[:])
                a2t = work.tile([P, 2, P], bf16, tag="a2t", name="a2t")
                for k in range(2):
                    pt = ps_l2.tile([P, P], bf16, tag="tps", name="pt")
                    nc.tensor.transpose(
                        out=pt[:], in_=a2[:, k * P:(k + 1) * P], identity=ident_b[:]
                    )
                    nc.vector.tensor_copy(out=a2t[:, k, :], in_=pt[:])
                pg = ps_l2.tile([P, D], fp32, tag="outg", name="pg")
                for k in range(2):
                    nc.tensor.matmul(
                        out=pg[:], lhsT=a2t[:, k, :], rhs=wg2_s[:, k, :],
                        start=(k == 0), stop=False,
                    )
                nc.tensor.matmul(
                    out=pg[:], lhsT=ones_row[:], rhs=bg2_s[:],
                    start=False, stop=True, skip_group_check=True,
                )
                nc.vector.tensor_copy(out=g_sball[:, ch, :], in_=pg[:])

            def txt_gather(r):
                # txtT_r[p, e, i] = g[slot=lidx[r*128+i]][e*128+p], straight
                # from SBUF: slot s -> partition s%128, rank s//128 = L2 chunk.
                txtT_r = work.tile([P, 4, P], bf16, tag="txtT", name="txtT_r",
                                   bufs=RT)
                nc.gpsimd.dma_gather(
                    out_ap=txtT_r[:], in_ap=g_sball[:],
                    idxs_ap=lidx_s[:, r * 8:(r + 1) * 8],
                    num_idxs=P, num_idxs_reg=P, elem_size=D,
                    transpose=True, single_packet=False, queue_num=r % 4,
                    sbuf_tokens_per_rank=P,
                    sbuf_free_dim_per_rank=D * 2,
                )
                return txtT_r

            contrib = stat.tile([P, RT], fp32)

            def logits_tile(r, txtT_r, ps_lg):
                # logits are bounded (|l| < ~10 for this model family), so
                # exp needs no max subtraction: lse = ln(sum(exp(l))).
                sums = stat.tile([P, NT], fp32, tag="sums", name="sums")
                diag = stat.tile([P, 1], fp32, tag="diag", name="diag")
                br, off = (r * P) // 512, (r * P) % 512
                for n in range(NT):
                    pl = ps_lg.tile([P, 512], fp32, tag="lg", name="pl", bufs=2)
                    for k in range(4):
                        nc.tensor.matmul(
                            out=pl[:], lhsT=txtT_r[:, k, :],
                            rhs=imgT_s[:, k, n * 512:(n + 1) * 512],
                            start=(k == 0), stop=(k == 3),
                        )
                    if n == br:
                        # diag block for row-tile r at local cols r*128..+127
                        dtmp = stat.tile([P, P], fp32, tag="dtmp", name="dtmp")
                        nc.vector.tensor_tensor(
                            out=dtmp[:], in0=pl[:, off:off + P],
                            in1=ident_f[:], op=mybir.AluOpType.mult,
                        )
                        nc.vector.reduce_sum(out=diag[:], in_=dtmp[:], axis=AX.X)
                    esc = work.tile([P, 512], bf16, tag="esc", name="esc", bufs=2)
                    nc.scalar.activation(
                        out=esc[:], in_=pl[:], func=AF.Exp,
                        scale=1.0, accum_out=sums[:, n:n + 1],
                    )
                ssum = stat.tile([P, 1], fp32, tag="ssum", name="ssum")
                nc.vector.reduce_sum(out=ssum[:], in_=sums[:], axis=AX.X)
                lns = stat.tile([P, 1], fp32, tag="lns", name="lns")
                nc.scalar.activation(out=lns[:], in_=ssum[:], func=AF.Ln)
                t1 = stat.tile([P, 1], fp32, tag="t1", name="t1")
                nc.vector.tensor_sub(out=t1[:], in0=lns[:], in1=diag[:])
                nc.vector.tensor_mul(
                    out=contrib[:, r:r + 1], in0=t1[:], in1=labf_s[:, r:r + 1]
                )

            # schedule: emit l2 chunks in order; logits row-tile r right after
            # chunk KR[r] (+1 lookahead chunk already emitted to hide latency)
            with tc.tile_pool(name="ps_l2", bufs=2, space="PSUM") as ps_l2, \
                 tc.tile_pool(name="ps_lg", bufs=1, space="PSUM") as ps_lg:
                r = 0
                for ch in range(LCH):
                    l2_chunk(ch, ps_l2)
                    while r < RT and KR[r] <= ch - 1:
                        logits_tile(r, txt_gather(r), ps_lg)
                        r += 1
                while r < RT:
                    logits_tile(r, txt_gather(r), ps_lg)
                    r += 1
            rsum = stat.tile([P, 1], fp32, tag="rsum")
            nc.vector.reduce_sum(out=rsum[:], in_=contrib[:], axis=AX.X)
            with tc.tile_pool(name="ps_fin", bufs=1, space="PSUM") as ps_fin:
                pf = ps_fin.tile([1, 1], fp32)
                nc.tensor.matmul(out=pf[:], lhsT=rsum[:], rhs=ones_col[:], start=True, stop=True)
                fin = stat.tile([1, 1], fp32, tag="fin")
                nc.vector.tensor_copy(out=fin[:], in_=pf[:])
            nc.sync.dma_start(out=t_out[:], in_=fin[:])

    nc.compile()
    return nc


_CACHE = {}


def kernel(**inputs) -> np.ndarray:
    from concourse.bass_utils import run_bass_kernel_spmd

    shared, percore, dims = _prep(inputs)
    key = tuple(sorted(dims.items()))
    if key not in _CACHE:
        _CACHE[key] = _build(**dims)
    nc = _CACHE[key]

    in_maps = []
    for c in range(NCORES):
        m = {
            "xt": shared["xt"], "wg1": shared["wg1"], "wg2": shared["wg2"],
            "wi1": shared["wi1"], "wi2": shared["wi2"],
            "bg1": shared["bg1"], "bg2": shared["bg2"],
            "bi1": shared["bi1"], "bi2": shared["bi2"],
            "cmat": percore[c]["cmat"], "gidx": percore[c]["gidx"],
            "cmat2": percore[c]["cmat2"], "gidx2": percore[c]["gidx2"],
            "lidx": percore[c]["lidx"],
            "labf": percore[c]["labf"], "imt": percore[c]["imt"],
        }
        in_maps.append(m)

    trace = bool(int(os.environ.get("KERNEL_TRACE", "0")))
    try:
        res = run_bass_kernel_spmd(nc, in_maps, core_ids=list(range(NCORES)), trace=trace)
    except Exception:
        # transient NRT/device hiccups have been observed to clear on retry
        res = run_bass_kernel_spmd(nc, in_maps, core_ids=list(range(NCORES)), trace=trace)
    kernel.last_results = res
    total = sum(float(r["partial"][0, 0]) for r in res.results)
    return np.float32(total / BATCH + 1.0)


# revision 62
# speedup vs baseline: 1.1838x; 1.1838x over previous
"""Trainium2 Bass kernel for nn_CLIP_GCN_Model (2-layer GCN + MLP + contrastive loss).

Reformulation (validated numerically): out = mean_i(label_i * (lse_i - logits_ii)) + 1.0
(the triplet term of the reference is identically 1.0).

GCN layer: out = S @ (x @ W) + b where S = D^-1/2 (A+I) D^-1/2.
  Layer 1: gather raw x rows (512-wide) per dst-chunk, aggregate via one-hot
    coefficient matmuls (C_j.T @ rows_j accumulated in PSUM), then apply W_g1.
  Layer 2: only the ~3.3k unique labeled nodes need gcn_out. Each core owns a
    balanced subset (LCH chunks of 128); gathers h rows (256-wide), aggregates,
    applies W_g2. Edges are split by AllGather-half of their src so the first
    gather of each chunk can start after AG half 1.

Sharding: 80 L1 dst-chunks assigned 10/core by LPT on edge counts (balances the
work in front of each AllGather). h AllGathered in 2 halves (bubble filled by
the image MLP). Labeled nodes assigned to cores by LPT on batch-row counts.
Contrastive row r lives on the core owning label_r; image columns permuted per
core so owned rows' diagonal lands at local col == local row. Gathers spread
round-robin over the 4 SWDGE queues (one Q7 core pair each) for 4x descriptor
emission concurrency.
"""

import os
import numpy as np
import ml_dtypes

BF16 = ml_dtypes.bfloat16

N_NODES = 10000
NPAD = 10240
D = 512
Hdim = 256
BATCH = 4096
NCORES = 8
P = 128
NCHUNK = NPAD // P          # 80
CPC = NCHUNK // NCORES      # 10 chunks per core
NPC = NPAD // NCORES        # 1280 nodes per core (h rows per core in h_t)
NT = BATCH // 512           # 8 column tiles of 512
CH1 = CPC // 2              # chunks before AllGather half 1


def _wrap16(idx, n):
    """Layout indices for dma_gather: element i -> [i%16, i//16], replicated to 128 partitions."""
    assert len(idx) == n and n % 16 == 0
    base = idx.astype(np.int16).reshape(n // 16, 16).T  # [16, n/16]
    return np.ascontiguousarray(np.tile(base, (8, 1)))  # [128, n/16]


def _lpt_assign(weights, n_bins, cap):
    """Greedy LPT: assign items (sorted by weight desc) to least-loaded bin with
    < cap items. Returns list of item-index lists per bin."""
    order = np.argsort(-np.asarray(weights), kind="stable")
    loads = [0.0] * n_bins
    counts = [0] * n_bins
    bins = [[] for _ in range(n_bins)]
    for it in order:
        best = min((b for b in range(n_bins) if counts[b] < cap),
                   key=lambda b: loads[b])
        bins[best].append(int(it))
        loads[best] += weights[it]
        counts[best] += 1
    return bins


def _prep(inputs):
    """Host-side layout/sharding prep. Returns (shared, percore, dims)."""
    x = np.ascontiguousarray(np.asarray(inputs["x_nodes"], dtype=np.float32))
    image = np.ascontiguousarray(np.asarray(inputs["image"], dtype=np.float32))
    ei = np.asarray(inputs["edge_index"]).astype(np.int64)
    label = np.asarray(inputs["label"]).astype(np.int64)
    src, dst = ei[0], ei[1]

    deg = np.ones(N_NODES, np.float32)
    np.add.at(deg, dst, 1.0)
    dinv = (1.0 / np.sqrt(deg)).astype(np.float32)

    # edges + self loops, sorted by dst
    src_all = np.concatenate([src, np.arange(N_NODES)])
    dst_all = np.concatenate([dst, np.arange(N_NODES)])
    coef_all = np.concatenate([dinv[src] * dinv[dst], dinv * dinv]).astype(np.float32)
    order = np.argsort(dst_all, kind="stable")
    src_s, dst_s, coef_s = src_all[order], dst_all[order], coef_all[order]

    counts = np.bincount(dst_s // P, minlength=NCHUNK)
    T_MAX = int(np.ceil(counts.max() / P))
    E_c = T_MAX * P
    starts = np.zeros(NCHUNK + 1, np.int64)
    np.cumsum(counts, out=starts[1:])

    # ---- L1 chunk -> core assignment, balanced by edge count ----
    chunk_bins = _lpt_assign(counts.astype(np.float64), NCORES, CPC)
    # remap: global node -> row in h_t ([half][core][pos%5][128])
    chunk_core = np.zeros(NCHUNK, np.int64)
    chunk_pos = np.zeros(NCHUNK, np.int64)
    for c in range(NCORES):
        for p_i, g in enumerate(chunk_bins[c]):
            chunk_core[g] = c
            chunk_pos[g] = p_i
    # single post-L1 AllGather: h_t rows are [core][pos][128]
    n_all = np.arange(NPAD)
    cg = n_all // P
    remap_row = chunk_core[cg] * NPC + chunk_pos[cg] * P + (n_all % P)

    # ---- per-core L1 gather/coefficient structures ----
    # raw-x 512-wide gathers (1KB rows drain ~2.4x faster than 512B rows).
    # Edges dedup by src within a chunk (the C row then has several dst cols).
    chunk_data = {}
    ucnt = np.zeros(NCHUNK, np.int64)
    for g in range(NCHUNK):
        e0, e1 = starts[g], starts[g + 1]
        usrc, inv = np.unique(src_s[e0:e1], return_inverse=True)
        chunk_data[g] = (usrc, inv, dst_s[e0:e1] - g * P, coef_s[e0:e1])
        ucnt[g] = len(usrc)
    T_MAX = int(np.ceil(ucnt.max() / P))
    E_c = T_MAX * P

    l1 = []
    for c in range(NCORES):
        gidx = np.zeros((CPC, E_c), np.int64)
        C = np.zeros((CPC, E_c, P), np.float32)
        for i, g in enumerate(chunk_bins[c]):
            usrc, inv, ld, cf = chunk_data[g]
            gidx[i, :len(usrc)] = usrc
            np.add.at(C[i], (inv, ld), cf)
        C = C.reshape(CPC, T_MAX, P, P).astype(BF16)
        l1.append({"gidx": gidx, "C": C})

    # ---- labeled-node ownership, balanced by batch-row count ----
    uniq, lab_inv, lab_rows = np.unique(label, return_inverse=True,
                                        return_counts=True)
    n_uniq = len(uniq)
    LCH = int(np.ceil(np.ceil(n_uniq / NCORES) / P))
    CAP = LCH * P
    node_bins = _lpt_assign(lab_rows.astype(np.float64), NCORES, CAP)
    LCH = int(np.ceil(max(len(b) for b in node_bins) / P))
    CAP = LCH * P

    # within each core, deal nodes round-robin by degree into LCH chunks so
    # per-chunk edge counts stay even (T2 is a global max)
    own_nodes = np.full((NCORES, CAP), -1, np.int64)   # global node id per slot
    owner_of = {}
    slot_of = {}
    for c in range(NCORES):
        items = node_bins[c]
        # true edge count of node u = edges with dst == u (computed below per
        # node is expensive; chunk-level degree is a fine proxy for dealing)
        degs = np.asarray([deg[uniq[i]] for i in items])
        items = [items[k] for k in np.argsort(-degs, kind="stable")]
        per_chunk = [[] for _ in range(LCH)]
        for k, it in enumerate(items):
            per_chunk[k % LCH].append(it)
        for ch in range(LCH):
            assert len(per_chunk[ch]) <= P
            for d, it in enumerate(per_chunk[ch]):
                u = int(uniq[it])
                own_nodes[c, ch * P + d] = u
                owner_of[u] = c
                slot_of[u] = ch * P + d

    # ---- per-core L2 structures (labeled chunks, src-half-split edges) ----
    # first pass: per (core, chunk) collect edges split by src half
    edgesA = [[[] for _ in range(LCH)] for _ in range(NCORES)]
    edgesB = [[[] for _ in range(LCH)] for _ in range(NCORES)]
    for c in range(NCORES):
        for s in range(CAP):
            u = own_nodes[c, s]
            if u < 0:
                continue
            ch, d = s // P, s % P
            e0, e1 = starts[u // P], starts[u // P + 1]
            sel = dst_s[e0:e1] == u
            esrc = src_s[e0:e1][sel]
            ecoef = coef_s[e0:e1][sel]
            rows = remap_row[esrc]
            for r, cf in zip(rows, ecoef):
                (edgesA if r < NPAD // 2 else edgesB)[c][ch].append((int(r), d, float(cf)))
    JA = max(1, int(np.ceil(max(len(edgesA[c][ch]) for c in range(NCORES)
                                for ch in range(LCH)) / P)))
    JB = max(1, int(np.ceil(max(len(edgesB[c][ch]) for c in range(NCORES)
                                for ch in range(LCH)) / P)))
    T2 = JA + JB
    E2 = T2 * P

    l2 = []
    for c in range(NCORES):
        # half-A rows index h_t_a directly; half-B rows are rebased to h_t_b
        # (row - NPAD//2). Padding rows stay 0 (valid in either half-table).
        gidx2 = np.zeros((LCH, E2), np.int64)
        ldst2 = np.zeros((LCH, E2), np.int64)
        cval2 = np.zeros((LCH, E2), np.float32)
        for ch in range(LCH):
            ea, eb = edgesA[c][ch], edgesB[c][ch]
            for k, (r, d, cf) in enumerate(ea):
                gidx2[ch, k] = r
                ldst2[ch, k] = d
                cval2[ch, k] = cf
            for k, (r, d, cf) in enumerate(eb):
                gidx2[ch, JA * P + k] = r - NPAD // 2
                ldst2[ch, JA * P + k] = d
                cval2[ch, JA * P + k] = cf
        C2 = np.zeros((LCH, T2, P, P), BF16)
        jj = np.arange(E2) // P
        pp = np.arange(E2) % P
        for ch in range(LCH):
            C2[ch, jj, pp, ldst2[ch]] = cval2[ch].astype(BF16)
        l2.append({"gidx2": gidx2, "C2": C2})

    xpad = np.zeros((NPAD, D), np.float32)
    xpad[:N_NODES] = x
    xrow = np.ascontiguousarray(xpad).astype(BF16)  # [10240, 512] row-major

    def km(w, kt):  # [K, M] -> [128p, kt, M]
        return np.ascontiguousarray(
            w.reshape(kt, P, w.shape[1]).transpose(1, 0, 2)
        ).astype(BF16)

    shared = {
        "xrow": xrow,
        "wg1": km(np.asarray(inputs["W_g1"], np.float32), 4),   # [128, 4, 256]
        "wg2": km(np.asarray(inputs["W_g2"], np.float32), 2),   # [128, 2, 512]
        "wi1": np.ascontiguousarray(
            np.asarray(inputs["W_img1"], np.float32).reshape(4, P, 2, P).transpose(1, 0, 2, 3)
        ).astype(BF16),                                         # [128, 4k, 2m, 128]
        "wi2": np.ascontiguousarray(
            np.asarray(inputs["W_img2"], np.float32).reshape(2, P, 4, P).transpose(1, 0, 2, 3)
        ).astype(BF16),                                         # [128, 2k, 4m, 128]
        "bg1": np.asarray(inputs["b_g1"], np.float32).astype(BF16).reshape(1, Hdim),
        "bg2": np.asarray(inputs["b_g2"], np.float32).astype(BF16).reshape(1, D),
        "bi1": np.ascontiguousarray(np.asarray(inputs["b_img1"], np.float32).reshape(2, P).T),
        "bi2": np.ascontiguousarray(np.asarray(inputs["b_img2"], np.float32).reshape(4, P).T),
    }

    # ---- contrastive rows: owner core of each batch row's label ----
    # rows ordered by the label's L2 chunk so logits row-tile r only needs
    # g_own chunks 0..KR[r] (enables L2/logits pipelining)
    owner = np.array([owner_of[int(l)] for l in label], np.int64)
    rows_by_core = []
    for c in range(NCORES):
        r = np.where(owner == c)[0]
        ch_of = np.array([slot_of[int(l)] // P for l in label[r]])
        rows_by_core.append(r[np.argsort(ch_of, kind="stable")])
    RT = max(2, int(np.ceil(max(len(r) for r in rows_by_core) / P)))
    ROWS = RT * P
    # KR[r] = max L2 chunk needed by logits row-tile r (global max over cores,
    # nondecreasing; pad rows use slot 0 = chunk 0)
    KR = [0] * RT
    for c in range(NCORES):
        slots = np.zeros(ROWS, np.int64)
        rl = rows_by_core[c]
        slots[:len(rl)] = [slot_of[int(l)] for l in label[rl]]
        for r in range(RT):
            KR[r] = max(KR[r], int(slots[r * P:(r + 1) * P].max()) // P)
    for r in range(1, RT):
        KR[r] = max(KR[r], KR[r - 1])
    KR = tuple(KR)

    percore = []
    imageb = image.astype(BF16)
    for c in range(NCORES):
        Cc = np.ascontiguousarray(
            l1[c]["C"].transpose(2, 0, 1, 3))          # [128p, 10, T_MAX, 128]
        gi = np.zeros((P, CPC, E_c // 16), np.int16)
        for i in range(CPC):
            gi[:, i, :] = _wrap16(l1[c]["gidx"][i], E_c)
        C2c = np.ascontiguousarray(
            l2[c]["C2"].transpose(2, 0, 1, 3))         # [128p, LCH, T2, 128]
        gi2 = np.zeros((P, LCH, E2 // 16), np.int16)
        for ch in range(LCH):
            gi2[:, ch, :] = _wrap16(l2[c]["gidx2"][ch], E2)

        rows = rows_by_core[c]
        n_c = len(rows)
        loc = np.zeros(ROWS, np.int64)
        loc[:n_c] = [slot_of[int(l)] for l in label[rows]]
        li = _wrap16(loc, ROWS)
        lab_f = np.zeros(ROWS, np.float32)
        lab_f[:n_c] = label[rows].astype(np.float32)
        lab_f = np.ascontiguousarray(lab_f.reshape(RT, P).T)   # [128, RT]
        rest = np.setdiff1d(np.arange(BATCH), rows, assume_unique=True)
        perm = np.concatenate([rows, rest])
        imr = imageb[perm]
        imt = np.ascontiguousarray(
            imr.reshape(NT, 512, 4, P).transpose(0, 3, 2, 1)
        )  # [8, 128p, 4k, 512w]
        percore.append({"cmat": Cc, "gidx": gi, "cmat2": C2c, "gidx2": gi2,
                        "lidx": li, "labf": lab_f, "imt": imt,
                        "own_nodes": own_nodes[c], "rows": rows})
    dims = dict(T_MAX=T_MAX, T2=T2, JA=JA, LCH=LCH, RT=RT, KR=KR)
    return shared, percore, dims


def _build(T_MAX, T2, JA, LCH, RT, KR):
    """Build the SPMD Bass program."""
    import concourse.bass as bass  # noqa: F401
    import concourse.tile as tile
    from concourse import bacc, mybir
    from concourse.masks import make_identity

    fp32 = mybir.dt.float32
    bf16 = mybir.dt.bfloat16
    i16 = mybir.dt.int16
    AF = mybir.ActivationFunctionType
    AX = mybir.AxisListType
    E_c = T_MAX * P
    E2 = T2 * P
    JB = T2 - JA
    ROWS = RT * P
    LROWS = LCH * P

    nc = bacc.Bacc("TRN2", target_bir_lowering=False, debug=False,
                   num_devices=NCORES, num_swdge_queues=4)

    t_xrow = nc.dram_tensor("xrow", [NPAD, D], bf16, kind="ExternalInput").ap()
    t_wg1 = nc.dram_tensor("wg1", [P, 4, Hdim], bf16, kind="ExternalInput").ap()
    t_wg2 = nc.dram_tensor("wg2", [P, 2, D], bf16, kind="ExternalInput").ap()
    t_wi1 = nc.dram_tensor("wi1", [P, 4, 2, P], bf16, kind="ExternalInput").ap()
    t_wi2 = nc.dram_tensor("wi2", [P, 2, 4, P], bf16, kind="ExternalInput").ap()
    t_bg1 = nc.dram_tensor("bg1", [1, Hdim], bf16, kind="ExternalInput").ap()
    t_bg2 = nc.dram_tensor("bg2", [1, D], bf16, kind="ExternalInput").ap()
    t_bi1 = nc.dram_tensor("bi1", [P, 2], fp32, kind="ExternalInput").ap()
    t_bi2 = nc.dram_tensor("bi2", [P, 4], fp32, kind="ExternalInput").ap()
    t_cmat = nc.dram_tensor("cmat", [P, CPC, T_MAX, P], bf16, kind="ExternalInput").ap()
    t_gidx = nc.dram_tensor("gidx", [P, CPC, E_c // 16], i16, kind="ExternalInput").ap()
    t_cmat2 = nc.dram_tensor("cmat2", [P, LCH, T2, P], bf16, kind="ExternalInput").ap()
    t_gidx2 = nc.dram_tensor("gidx2", [P, LCH, E2 // 16], i16, kind="ExternalInput").ap()
    t_lidx = nc.dram_tensor("lidx", [P, ROWS // 16], i16, kind="ExternalInput").ap()
    t_labf = nc.dram_tensor("labf", [P, RT], fp32, kind="ExternalInput").ap()
    t_imt = nc.dram_tensor("imt", [NT, P, 4, 512], bf16, kind="ExternalInput").ap()
    t_out = nc.dram_tensor("partial", [1, 1], fp32, kind="ExternalOutput").ap()

    rg = [list(range(NCORES))]

    with tile.TileContext(nc) as tc:
        from contextlib import ExitStack
        with ExitStack() as ctx:
            dram = ctx.enter_context(tc.tile_pool(name="dram", bufs=1, space="DRAM"))
            const = ctx.enter_context(tc.tile_pool(name="const", bufs=1))
            big = ctx.enter_context(tc.tile_pool(name="big", bufs=1))
            work = ctx.enter_context(tc.tile_pool(name="work", bufs=3))
            gbuf = ctx.enter_context(tc.tile_pool(name="gbuf", bufs=2))
            cbuf = ctx.enter_context(tc.tile_pool(name="cbuf", bufs=3))
            stat = ctx.enter_context(tc.tile_pool(name="stat", bufs=4))

            h_own = dram.tile([NPC, Hdim], bf16)
            h_t = dram.tile([NPAD, Hdim], bf16, addr_space="Shared")

            # ---- constants in SBUF (gather indices first: gathers need them) ----
            gidx_s = const.tile([P, CPC, E_c // 16], i16)
            nc.sync.dma_start(out=gidx_s[:], in_=t_gidx[:])

            imgT_s = big.tile([P, 4, BATCH], bf16)  # transposed img (permuted cols)
            g_sball = big.tile([P, LCH, D], bf16)   # gcn_out: slot s -> [s%128, s//128, :]

            # ---- constants ----
            wg1_s = const.tile([P, 4, Hdim], bf16)
            nc.sync.dma_start(out=wg1_s[:], in_=t_wg1[:])
            bg1_s = const.tile([1, Hdim], bf16)
            nc.sync.dma_start(out=bg1_s[:], in_=t_bg1[:])
            wi1_s = const.tile([P, 4, 2, P], bf16)
            nc.scalar.dma_start(out=wi1_s[:], in_=t_wi1[:])
            wi2_s = const.tile([P, 2, 4, P], bf16)
            nc.scalar.dma_start(out=wi2_s[:], in_=t_wi2[:])
            bi1_s = const.tile([P, 2], fp32)
            nc.scalar.dma_start(out=bi1_s[:], in_=t_bi1[:])
            bi2_s = const.tile([P, 4], fp32)
            nc.scalar.dma_start(out=bi2_s[:], in_=t_bi2[:])
            wg2_s = const.tile([P, 2, D], bf16)
            nc.sync.dma_start(out=wg2_s[:], in_=t_wg2[:])
            bg2_s = const.tile([1, D], bf16)
            nc.sync.dma_start(out=bg2_s[:], in_=t_bg2[:])
            gidx2_s = const.tile([P, LCH, E2 // 16], i16)
            nc.sync.dma_start(out=gidx2_s[:], in_=t_gidx2[:])
            lidx_s = const.tile([P, ROWS // 16], i16)
            nc.sync.dma_start(out=lidx_s[:], in_=t_lidx[:])
            labf_s = const.tile([P, RT], fp32)
            nc.sync.dma_start(out=labf_s[:], in_=t_labf[:])
            ones_row = const.tile([1, P], bf16)
            nc.vector.memset(ones_row[:], 1.0)
            ones_col = const.tile([P, 1], fp32)
            nc.vector.memset(ones_col[:], 1.0)
            ident_b = const.tile([P, P], bf16)
            make_identity(nc, ident_b[:])
            ident_f = const.tile([P, P], fp32)
            make_identity(nc, ident_f[:])

            # ===== L1 prefetch: cm load (sync) + half-split 1KB-row gathers ==
            TH = T_MAX // 2
            HALF = ((0, TH), (TH, T_MAX))
            fetched = {}

            def l1_fetch(i):
                cm = cbuf.tile([P, T_MAX, P], bf16, tag="cm", name="cm")
                nc.sync.dma_start(out=cm[:], in_=t_cmat[:, i, :, :])
                ghs = []
                for hf, (j0, j1) in enumerate(HALF):
                    gh = gbuf.tile([P, j1 - j0, D], bf16, tag=f"g1_{hf}", name="gh")
                    nc.gpsimd.dma_gather(
                        out_ap=gh[:], in_ap=t_xrow[:, :],
                        idxs_ap=gidx_s[:, i, j0 * 8:j1 * 8],
                        num_idxs=(j1 - j0) * P, num_idxs_reg=(j1 - j0) * P,
                        elem_size=D, single_packet=False,
                        queue_num=(2 * i + hf) % 4,
                    )
                    ghs.append(gh)
                fetched[i] = (cm, ghs)

            l1_fetch(0)
            l1_fetch(1)

            # ---- image MLP tile (used at start for PE warmup + AG2 bubble) ----
            def mlp_tile(n, ps_mlp):
                imt_n = work.tile([P, 4, 512], bf16, tag="imt_n", name="imt_n", bufs=2)
                nc.scalar.dma_start(out=imt_n[:], in_=t_imt[n])
                h1t = work.tile([P, 2, 512], bf16, tag="h1t", name="h1t")
                for m in range(2):
                    pm = ps_mlp.tile([P, 512], fp32, tag="mlp1", name="pm")
                    for k in range(4):
                        nc.tensor.matmul(
                            out=pm[:], lhsT=wi1_s[:, k, m, :], rhs=imt_n[:, k, :],
                            start=(k == 0), stop=(k == 3),
                        )
                    nc.scalar.activation(
                        out=h1t[:, m, :], in_=pm[:], func=AF.Relu,
                        bias=bi1_s[:, m:m + 1], scale=1.0,
                    )
                for m in range(4):
                    pm2 = ps_mlp.tile([P, 512], fp32, tag="mlp2", name="pm2")
                    for k in range(2):
                        nc.tensor.matmul(
                            out=pm2[:], lhsT=wi2_s[:, k, m, :], rhs=h1t[:, k, :],
                            start=(k == 0), stop=(k == 1),
                        )
                    nc.scalar.activation(
                        out=imgT_s[:, m, n * 512:(n + 1) * 512], in_=pm2[:],
                        func=AF.Relu, bias=bi2_s[:, m:m + 1], scale=1.0,
                    )

            # ===== GCN layer 1 compute (consumes prefetched cm + gathers) =====
            def l1_compute(i, ps_ag):
                cm, ghs = fetched.pop(i)
                pa = ps_ag.tile([P, D], fp32, tag="agg1", name="pa")
                for hf, (j0, j1) in enumerate(HALF):
                    for j in range(j0, j1):
                        nc.tensor.matmul(
                            out=pa[:], lhsT=cm[:, j, :], rhs=ghs[hf][:, j - j0, :],
                            start=(j == 0), stop=(j == T_MAX - 1),
                        )
                a1 = work.tile([P, D], bf16, tag="a1", name="a1")
                nc.vector.tensor_copy(out=a1[:], in_=pa[:])
                a1t = work.tile([P, 4, P], bf16, tag="a1t", name="a1t")
                for k in range(4):
                    pt1 = ps_ag.tile([P, P], bf16, tag="tps1", name="pt1")
                    nc.tensor.transpose(
                        out=pt1[:], in_=a1[:, k * P:(k + 1) * P], identity=ident_b[:]
                    )
                    nc.vector.tensor_copy(out=a1t[:, k, :], in_=pt1[:])
                ph = ps_ag.tile([P, Hdim], fp32, tag="hps", name="ph")
                for k in range(4):
                    nc.tensor.matmul(
                        out=ph[:], lhsT=a1t[:, k, :], rhs=wg1_s[:, k, :],
                        start=(k == 0), stop=False,
                    )
                nc.tensor.matmul(
                    out=ph[:], lhsT=ones_row[:], rhs=bg1_s[:],
                    start=False, stop=True, skip_group_check=True,
                )
                h_sb = work.tile([P, Hdim], bf16, tag="h_sb", name="h_sb")
                nc.vector.tensor_scalar_max(out=h_sb[:], in0=ph[:], scalar1=0.0)
                nc.sync.dma_start(out=h_own[i * P:(i + 1) * P, :], in_=h_sb[:])

            with tc.tile_pool(name="ps_ag", bufs=2, space="PSUM") as ps_ag:
                for i in range(CPC):
                    if i + 2 < CPC:
                        l1_fetch(i + 2)
                    l1_compute(i, ps_ag)
                nc.gpsimd.collective_compute(
                    "AllGather", mybir.AluOpType.bypass, replica_groups=rg,
                    ins=[h_own[:, :]], outs=[h_t[:, :]],
                )

            # the whole image MLP fills the AllGather bubble and keeps the
            # PE warm into the L2/logits tail
            with tc.tile_pool(name="ps_mlp", bufs=2, space="PSUM") as ps_mlp:
                for n in range(NT):
                    mlp_tile(n, ps_mlp)

            # ====== L2 gathers: early halves (need AG1) then late (need AG2) ==
            # emitted on the gpsimd queue before the MLP tensor work so the Q7
            # pairs stay busy through the AG2 bubble.
            g2e, g2l, cm2s = [], [], []
            for ch in range(LCH):
                cm2 = cbuf.tile([P, T2, P], bf16, tag="cm2", name="cm2", bufs=LCH)
                nc.sync.dma_start(out=cm2[:], in_=t_cmat2[:, ch, :, :])
                cm2s.append(cm2)
            for ch in range(LCH):
                ge = gbuf.tile([P, JA, Hdim], bf16, tag="g2e", name="ge", bufs=LCH)
                nc.gpsimd.dma_gather(
                    out_ap=ge[:], in_ap=h_t[0:NPAD // 2, :],
                    idxs_ap=gidx2_s[:, ch, 0:JA * 8],
                    num_idxs=JA * P, num_idxs_reg=JA * P,
                    elem_size=Hdim, single_packet=False,
                    queue_num=ch % 4,
                )
                g2e.append(ge)
            for ch in range(LCH):
                gl = gbuf.tile([P, JB, Hdim], bf16, tag="g2l", name="gl", bufs=LCH)
                nc.gpsimd.dma_gather(
                    out_ap=gl[:], in_ap=h_t[NPAD // 2:NPAD, :],
                    idxs_ap=gidx2_s[:, ch, JA * 8:T2 * 8],
                    num_idxs=JB * P, num_idxs_reg=JB * P,
                    elem_size=Hdim, single_packet=False,
                    queue_num=(ch + LCH) % 4,
                )
                g2l.append(gl)

            # ====== GCN layer 2 + txt + logits, pipelined per KR schedule =====
            def l2_chunk(ch, ps_l2):
                pa2 = ps_l2.tile([P, Hdim], fp32, tag="agg2", name="pa2")
                for j in range(JA):
                    nc.tensor.matmul(
                        out=pa2[:], lhsT=cm2s[ch][:, j, :], rhs=g2e[ch][:, j, :],
                        start=(j == 0), stop=False,
                    )
                for j in range(JB):
                    nc.tensor.matmul(
                        out=pa2[:], lhsT=cm2s[ch][:, JA + j, :], rhs=g2l[ch][:, j, :],
                        start=False, stop=(j == JB - 1),
                    )
                a2 = work.tile([P, Hdim], bf16, tag="a2", name="a2")
                nc.vector.tensor_copy(out=a2[:], in_=pa2[:])
                a2t = work.tile([P, 2, P], bf16, tag="a2t", name="a2t")
                for k in range(2):
                    pt = ps_l2.tile([P, P], bf16, tag="tps", name="pt")
                    nc.tensor.transpose(
                        out=pt[:], in_=a2[:, k * P:(k + 1) * P], identity=ident_b[:]
                    )
                    nc.vector.tensor_copy(out=a2t[:, k, :], in_=pt[:])
                pg = ps_l2.tile([P, D], fp32, tag="outg", name="pg")
                for k in range(2):
                    nc.tensor.matmul(
                        out=pg[:], lhsT=a2t[:, k, :], rhs=wg2_s[:, k, :],
                        start=(k == 0), stop=False,
                    )
                nc.tensor.matmul(
                    out=pg[:], lhsT=ones_row[:], rhs=bg2_s[:],
                    start=False, stop=True, skip_group_check=True,
                )
                nc.vector.tensor_copy(out=g_sball[:, ch, :], in_=pg[:])

            def txt_gather(r):
                # txtT_r[p, e, i] = g[slot=lidx[r*128+i]][e*128+p], straight
                # from SBUF: slot s -> partition s%128, rank s//128 = L2 chunk.
                txtT_r = work.tile([P, 4, P], bf16, tag="txtT", name="txtT_r",
                                   bufs=RT)
                nc.gpsimd.dma_gather(
                    out_ap=txtT_r[:], in_ap=g_sball[:],
                    idxs_ap=lidx_s[:, r * 8:(r + 1) * 8],
                    num_idxs=P, num_idxs_reg=P, elem_size=D,
                    transpose=True, single_packet=False, queue_num=r % 4,
                    sbuf_tokens_per_rank=P,
                    sbuf_free_dim_per_rank=D * 2,
                )
                return txtT_r

            contrib = stat.tile([P, RT], fp32)

            def logits_tile(r, txtT_r, ps_lg):
                # logits are bounded (|l| < ~10 for this model family), so
                # exp needs no max subtraction: lse = ln(sum(exp(l))).
                sums = stat.tile([P, NT], fp32, tag="sums", name="sums")
                diag = stat.tile([P, 1], fp32, tag="diag", name="diag")
                br, off = (r * P) // 512, (r * P) % 512
                for n in range(NT):
                    pl = ps_lg.tile([P, 512], fp32, tag="lg", name="pl", bufs=2)
                    for k in range(4):
                        nc.tensor.matmul(
                            out=pl[:], lhsT=txtT_r[:, k, :],
                            rhs=imgT_s[:, k, n * 512:(n + 1) * 512],
                            start=(k == 0), stop=(k == 3),
                        )
                    if n == br:
                        # diag block for row-tile r at local cols r*128..+127
                        dtmp = stat.tile([P, P], fp32, tag="dtmp", name="dtmp")
                        nc.vector.tensor_tensor(
                            out=dtmp[:], in0=pl[:, off:off + P],
                            in1=ident_f[:], op=mybir.AluOpType.mult,
                        )
                        nc.vector.reduce_sum(out=diag[:], in_=dtmp[:], axis=AX.X)
                    esc = work.tile([P, 512], bf16, tag="esc", name="esc", bufs=2)
                    nc.scalar.activation(
                        out=esc[:], in_=pl[:], func=AF.Exp,
                        scale=1.0, accum_out=sums[:, n:n + 1],
                    )
                ssum = stat.tile([P, 1], fp32, tag="ssum", name="ssum")
                nc.vector.reduce_sum(out=ssum[:], in_=sums[:], axis=AX.X)
                lns = stat.tile([P, 1], fp32, tag="lns", name="lns")
                nc.scalar.activation(out=lns[:], in_=ssum[:], func=AF.Ln)
                t1 = stat.tile([P, 1], fp32, tag="t1", name="t1")
                nc.vector.tensor_sub(out=t1[:], in0=lns[:], in1=diag[:])
                nc.vector.tensor_mul(
                    out=contrib[:, r:r + 1], in0=t1[:], in1=labf_s[:, r:r + 1]
                )

            # schedule: emit l2 chunks in order; logits row-tile r right after
            # chunk KR[r] (+1 lookahead chunk already emitted to hide latency)
            with tc.tile_pool(name="ps_l2", bufs=2, space="PSUM") as ps_l2, \
                 tc.tile_pool(name="ps_lg", bufs=1, space="PSUM") as ps_lg:
                r = 0
                for ch in range(LCH):
                    l2_chunk(ch, ps_l2)
                    while r < RT and KR[r] <= ch - 1:
                        logits_tile(r, txt_gather(r), ps_lg)
                        r += 1
                while r < RT:
                    logits_tile(r, txt_gather(r), ps_lg)
                    r += 1
            rsum = stat.tile([P, 1], fp32, tag="rsum")
            nc.vector.reduce_sum(out=rsum[:], in_=contrib[:], axis=AX.X)
            with tc.tile_pool(name="ps_fin", bufs=1, space="PSUM") as ps_fin:
                pf = ps_fin.tile([1, 1], fp32)
                nc.tensor.matmul(out=pf[:], lhsT=rsum[:], rhs=ones_col[:], start=True, stop=True)
                fin = stat.tile([1, 1], fp32, tag="fin")
                nc.vector.tensor_copy(out=fin[:], in_=pf[:])
            nc.sync.dma_start(out=t_out[:], in_=fin[:])

    nc.compile()
    return nc


_CACHE = {}


def kernel(**inputs) -> np.ndarray:
    from concourse.bass_utils import run_bass_kernel_spmd

    shared, percore, dims = _prep(inputs)
    key = tuple(sorted(dims.items()))
    if key not in _CACHE:
        _CACHE[key] = _build(**dims)
    nc = _CACHE[key]

    in_maps = []
    for c in range(NCORES):
        m = {
            "xrow": shared["xrow"], "wg1": shared["wg1"], "wg2": shared["wg2"],
            "wi1": shared["wi1"], "wi2": shared["wi2"],
            "bg1": shared["bg1"], "bg2": shared["bg2"],
            "bi1": shared["bi1"], "bi2": shared["bi2"],
            "cmat": percore[c]["cmat"], "gidx": percore[c]["gidx"],
            "cmat2": percore[c]["cmat2"], "gidx2": percore[c]["gidx2"],
            "lidx": percore[c]["lidx"],
            "labf": percore[c]["labf"], "imt": percore[c]["imt"],
        }
        in_maps.append(m)

    trace = bool(int(os.environ.get("KERNEL_TRACE", "0")))
    try:
        res = run_bass_kernel_spmd(nc, in_maps, core_ids=list(range(NCORES)), trace=trace)
    except Exception:
        # transient NRT/device hiccups have been observed to clear on retry
        res = run_bass_kernel_spmd(nc, in_maps, core_ids=list(range(NCORES)), trace=trace)
    kernel.last_results = res
    total = sum(float(r["partial"][0, 0]) for r in res.results)
    return np.float32(total / BATCH + 1.0)
